# revision 1
# baseline (speedup 1.0000x reference)
"""Binarized dense layer for Trainium2 (8 NeuronCores, data-parallel).

Computes y = sign(x) @ sign(w) + b  with sign(v) = -1 if v < 0 else +1,
matching jnp.where(v < 0, -1, 1) bit-exactly (including v == +0.0 -> +1).

Full shapes: x [8192, 2048] f32, w [2048, 2048] f32, b [2048] f32
-> y [8192, 2048] f32. Rows of x are sharded across 8 cores; w, b are
replicated. Per-core kernel design:

  X path: DMA f32 row-chunks -> PE transpose-mode (128x128 f32 blocks,
      4 per PSUM bank) -> ScalarE Sign (+1e-30 bias so sign(0)=+1)
      evacuates to fp8e4 +-1 tiles in [k-partition, k-subtile, m] layout.
  W path: streamed by 512-column quarters (so each PSUM accumulation
      group's weights arrive k-complete early): DMA f32 -> ScalarE Sign
      -> fp8e4 quad tiles. Each quarter's bias slice is DMA-broadcast
      alongside it.
  Matmul: fp8 DoubleRow (256-row contraction per instruction; +-1
      products exact, fp32 PSUM accumulation, |sums| <= 2048 exact).
      8 accumulating matmuls per (m-tile, quarter) into one PSUM bank.
      The (quarter, m-tile) schedule interleaves q0/q1 m-blocks around
      the X stream to avoid PE FIFO head-of-line stalls on late X tiles.
  Epilogue: VectorE tensor_tensor adds the partition-replicated f32 bias
      (rounding matches the reference exactly); output DMAs issue from
      GPSIMD (SWDGE) so they never block input loads on the Sync queue.
"""
import numpy as np


import concourse.bass as bass
import concourse.mybir as mybir
import concourse.tile as tile
from concourse import bacc
from concourse.masks import make_identity

F32 = mybir.dt.float32
FP8 = mybir.dt.float8e4
P = 128
NQT = 512
Sign = mybir.ActivationFunctionType.Sign


def _build_kernel(M=1024, K=2048, N=2048, n_cores=8, xstage_bufs=5, wstage_bufs=6,
                 wq_bufs=3, tpsum_bufs=2, opsum_bufs=4, osb_bufs=4, tg=4,
                 out_eng='gpsimd', split=5, bias_eng='gpsimd', bias_early=True,
                 phase_barrier=False):
    KS = K // P
    KP = KS // 2
    MT = M // P
    NQ = N // NQT
    WG = 4
    NQUAD = KS // WG
    XQ = KS // tg            # xbt quads per m-tile
    nc = bacc.Bacc("TRN2", target_bir_lowering=False, debug=False, num_devices=n_cores)
    x = nc.dram_tensor("x", [M, K], F32, kind="ExternalInput").ap()
    w = nc.dram_tensor("w", [K, N], F32, kind="ExternalInput").ap()
    b = nc.dram_tensor("b", [N], F32, kind="ExternalInput").ap()
    y = nc.dram_tensor("y", [M, N], F32, kind="ExternalOutput").ap()
    w_r = w.rearrange("(a p) n -> p a n", p=P)

    with tile.TileContext(nc) as tc:
        with (
            tc.tile_pool(name="cst", bufs=1) as cst,
            tc.tile_pool(name="xstage", bufs=xstage_bufs) as xstage,
            tc.tile_pool(name="xbt", bufs=1) as xbtp,
            tc.tile_pool(name="wstage", bufs=wstage_bufs) as wstage,
            tc.tile_pool(name="wq", bufs=wq_bufs) as wqp,
            tc.tile_pool(name="osb", bufs=osb_bufs) as osbp,
            tc.tile_pool(name="tpsum", bufs=tpsum_bufs, space="PSUM") as tpsum,
            tc.tile_pool(name="opsum", bufs=opsum_bufs, space="PSUM") as opsum,
        ):
            eps = cst.tile([P, 1], F32, tag="eps")
            nc.vector.memset(eps[:], 1e-30)
            ident = cst.tile([P, P], F32, tag="ident")
            make_identity(nc, ident[:])
            bias_q = [cst.tile([P, NQT], F32, tag=f"bias{q}", name=f"bias{q}")
                      for q in range(NQ)]

            xbt = [[xbtp.tile([P, tg, P], FP8, tag=f"xbt{mi}_{g}",
                              name=f"xbt{mi}_{g}") for g in range(XQ)]
                   for mi in range(MT)]

            def load_x(mi):
                xs = xstage.tile([P, K], F32, tag="xs", name=f"xs{mi}")
                nc.sync.dma_start(xs[:], x[mi * P:(mi + 1) * P, :])
                return xs

            last_prep = [None]

            def prep_x(mi, xs):
                for g in range(XQ):
                    pt = tpsum.tile([P, tg * P], F32, tag="tp", name=f"tp{mi}_{g}")
                    for j in range(tg):
                        kj = g * tg + j
                        nc.tensor.transpose(pt[:, j * P:(j + 1) * P],
                                            xs[:, kj * P:(kj + 1) * P], ident[:])
                    last_prep[0] = nc.scalar.activation(
                        xbt[mi][g][:],
                        pt[:].rearrange("p (a m) -> p a m", a=tg),
                        Sign, bias=eps[:])

            def lhs_pair(mi, t):
                g, h = (2 * t) // tg, (2 * t) % tg
                return xbt[mi][g][:, h:h + 2, :]

            def load_wq(q):
                nc.sync.dma_start(
                    bias_q[q][:],
                    b[None, q * NQT:(q + 1) * NQT].to_broadcast([P, NQT]))
                quads = []
                for g in range(NQUAD):
                    ws = wstage.tile([P, WG, NQT], F32, tag="ws", name=f"ws{q}_{g}")
                    nc.sync.dma_start(
                        ws[:], w_r[:, g * WG:(g + 1) * WG,
                                   q * NQT:(q + 1) * NQT])
                    wqt = wqp.tile([P, WG, NQT], FP8, tag=f"wqt{g}",
                                   name=f"wq{q}_{g}")
                    nc.scalar.activation(wqt[:], ws[:], Sign, bias=eps[:])
                    quads.append(wqt)
                return quads

            def rhs_pair(quads, t):
                g, h = t // (WG // 2), t % (WG // 2)
                return quads[g][:, 2 * h:2 * h + 2, :]

            xs0 = load_x(0)
            wq_tiles = {0: load_wq(0)}
            prep_x(0, xs0)
            for mi in range(1, MT):
                xs = load_x(mi)
                if mi == min(2, MT - 1) and NQ > 1:
                    wq_tiles[1] = load_wq(1)
                prep_x(mi, xs)
            if 1 not in wq_tiles and NQ > 1:
                wq_tiles[1] = load_wq(1)

            # schedule: interleave q0/q1 around the X stream, then q2, q3
            if NQ >= 2 and MT > split:
                sched = [(0, mi) for mi in range(split)]
                sched += [(1, mi) for mi in range(split)]
                sched += [(0, mi) for mi in range(split, MT)]
                sched += [(1, mi) for mi in range(split, MT)]
                for q in range(2, NQ):
                    sched += [(q, mi) for mi in range(MT)]
            else:
                sched = [(q, mi) for q in range(NQ) for mi in range(MT)]
            prefetch_at = {}
            if NQ > 2:
                # emit load_wq(q+2) when q first appears in sched
                seen = set()
                for idx, (q, mi) in enumerate(sched):
                    if q not in seen:
                        seen.add(q)
                        if q + 2 < NQ:
                            prefetch_at[idx] = q + 2

            for idx, (q, mi) in enumerate(sched):
                if idx in prefetch_at:
                    wq_tiles[prefetch_at[idx]] = load_wq(prefetch_at[idx])
                quads = wq_tiles[q]
                op = opsum.tile([P, NQT], F32, tag="op", name=f"op{mi}_{q}")
                for t in range(KP):
                    h = nc.tensor.matmul(
                        op[:],
                        lhsT=lhs_pair(mi, t),
                        rhs=rhs_pair(quads, t),
                        start=(t == 0), stop=(t == KP - 1),
                        perf_mode=mybir.MatmulPerfMode.DoubleRow)
                    if phase_barrier and idx == 0 and t == 0 and last_prep[0] is not None:
                        from concourse.tile import add_dep_helper
                        add_dep_helper(h.ins, last_prep[0].ins, sync=True,
                                       reason="phase barrier: MMs after X prep")
                ob = osbp.tile([P, NQT], F32, tag="ob", name=f"ob{mi}_{q}")
                nc.vector.tensor_add(ob[:], op[:], bias_q[q][:])
                getattr(nc, out_eng).dma_start(
                    y[mi * P:(mi + 1) * P, q * NQT:(q + 1) * NQT], ob[:])
    nc.compile()
    return nc


N_CORES = 8
M_FULL, K_DIM, N_DIM = 8192, 2048, 2048
M_LOC = M_FULL // N_CORES
_nc_cache = {}


def _get_nc():
    if "nc" not in _nc_cache:
        _nc_cache["nc"] = _build_kernel(M=M_LOC, K=K_DIM, N=N_DIM,
                                        n_cores=N_CORES, split=6)
    return _nc_cache["nc"]


def kernel(inputs: np.ndarray, kernel: np.ndarray, bias: np.ndarray) -> np.ndarray:
    assert inputs.shape == (M_FULL, K_DIM) and inputs.dtype == np.float32
    assert kernel.shape == (K_DIM, N_DIM) and kernel.dtype == np.float32
    assert bias.shape == (N_DIM,) and bias.dtype == np.float32
    nc = _get_nc()
    in_maps = [
        {"x": inputs[c * M_LOC:(c + 1) * M_LOC, :], "w": kernel, "b": bias}
        for c in range(N_CORES)
    ]
    try:
        from concourse.bass_utils import run_bass_kernel_spmd
        results = run_bass_kernel_spmd(
            nc, in_maps, core_ids=list(range(N_CORES))).results
    except Exception:
        from concourse import bass2jax
        bass2jax.install_neuronx_cc_hook()
        results = bass2jax.run_bass_via_pjrt(nc, in_maps, n_cores=N_CORES)
    return np.concatenate([r["y"] for r in results], axis=0)

